# revision 1
# baseline (speedup 1.0000x reference)
"""Trainium2 Bass kernel: BERT-style self-attention with granularity-gated
sparse penalties (softmax(QK^T/sqrt(d) + log(penalties)) @ V).

Math restructure (exact up to ~1e-24 relative):
  softmax(S + log(max(pen, 1e-32))) == pen * exp(S) / sum_j(pen * exp(S))
  - no log needed, no max-subtraction (S bounded ~|25|, exp can't overflow)
  - masked entries (scope clipped at 0 instead of 1e-32) contribute 0

Layout: S^T tiles [128 keys x 512 queries] so the softmax reduction over keys
is a matmul contraction: l = ones-row folded into V_aug's 65th column.

Sharding: core c in 0..7 -> batch b=c//4, query slab q0=(c%4)*512, all 16
heads, all 2048 keys. Penalties [2048k x 512q] computed once per core in SBUF
(bf16), reused by all 16 heads.

The granularity gate g (a [B,S] vector, 0.02% of total FLOPs) is computed
host-side in f64 during input prep; the device receives the per-query /
per-key gate-derived vectors directly, so penalty computation starts at t=0
on the vector engine while projections run on the PE.

Precision: fp16 for hidden/W/Q/K (score path), bf16 for V/E/pen (exp values
exceed fp16 range), f32 PSUM accumulation everywhere.
"""

import math

import ml_dtypes
import numpy as np

import concourse.bass as bass
import concourse.tile as tile
from concourse import bacc, mybir
from concourse.bass import AP
from concourse.bass_utils import run_bass_kernel_spmd

F32 = mybir.dt.float32
BF16 = mybir.dt.bfloat16
FP16 = mybir.dt.float16
AF = mybir.ActivationFunctionType
OP = mybir.AluOpType

B, S, H = 2, 2048, 1024
NH, HD = 16, 64
NC = 8
SLAB = S // 4          # 512 queries per core
KT = S // 128          # 16 key tiles
LN_BASE = float(np.log(np.float32(S - 2)))  # ln(2046)
VW = HD + 1            # 65: V columns + ones column per (kt, head)


def build_nc():
    nc = bacc.Bacc("TRN2", target_bir_lowering=False, debug=False)

    hT = nc.dram_tensor("hT", [H, S], FP16, kind="ExternalInput").ap()
    hTq = nc.dram_tensor("hTq", [H, SLAB], FP16, kind="ExternalInput").ap()
    Wq = nc.dram_tensor("Wq", [8, H + 1, 128], FP16, kind="ExternalInput").ap()
    Wk = nc.dram_tensor("Wk", [8, H + 1, 128], FP16, kind="ExternalInput").ap()
    Wv = nc.dram_tensor("Wv", [2, H + 1, 512], FP16, kind="ExternalInput").ap()
    bqv = nc.dram_tensor("bqv", [H], F32, kind="ExternalInput").ap()
    bkv = nc.dram_tensor("bkv", [H], F32, kind="ExternalInput").ap()
    bvp = nc.dram_tensor("bvp", [VW, NH], F32, kind="ExternalInput").ap()
    idx = nc.dram_tensor("idx", [S], F32, kind="ExternalInput").ap()
    zk = nc.dram_tensor("zk", [S], F32, kind="ExternalInput").ap()
    qv3 = nc.dram_tensor("qv3", [3 * SLAB], BF16, kind="ExternalInput").ap()
    qv2 = nc.dram_tensor("qv2", [2 * SLAB], F32, kind="ExternalInput").ap()
    out = nc.dram_tensor("out", [NH * VW, SLAB], F32, kind="ExternalOutput").ap()

    with tile.TileContext(nc) as tc:
        _body(tc, nc, hT, hTq, Wq, Wk, Wv, bqv, bkv, bvp, idx, zk,
              qv3, qv2, out)
    nc.compile()
    return nc


def _bcast(ap_1d, n_part, n_free):
    """[n] dram AP -> [n_part, n_free] partition-broadcast AP."""
    return AP(tensor=ap_1d.tensor, offset=ap_1d.offset, ap=[[0, n_part], [1, n_free]])


def _pm_view(ap_1d, n_free):
    """[128*n_free] dram AP <-> [128, n_free] partition-major SBUF tile:
    element (p, f) maps to dram[p + 128*f]."""
    return AP(tensor=ap_1d.tensor, offset=ap_1d.offset, ap=[[1, 128], [128, n_free]])



def _wview(w3d, idx, width):
    """Wx[idx] ([1025, width] row-major) as a [128, 8, width] AP:
    (p, ht, c) <- dram row ht*128+p, col c."""
    return AP(
        tensor=w3d.tensor,
        offset=w3d.offset + idx * (H + 1) * width,
        ap=[[width, 128], [128 * width, 8], [1, width]],
    )


def _body(tc, nc, hT, hTq, Wq, Wk, Wv, bqv, bkv, bvp, idx, zk,
          qv3, qv2, out):
    import contextlib

    ctx = contextlib.ExitStack()
    with ctx:
        big = ctx.enter_context(tc.tile_pool(name="big", bufs=1))
        consts = ctx.enter_context(tc.tile_pool(name="consts", bufs=1))
        wk_pool = ctx.enter_context(tc.tile_pool(name="wk", bufs=2))
        wv_pool = ctx.enter_context(tc.tile_pool(name="wv", bufs=1))

        # --- resident SBUF tensors ---
        # one tile per h-chunk so matmuls depend only on their own DMA
        hT_ts = [big.tile([128, S], FP16, name=f"hT{i}", tag=f"hT{i}") for i in range(8)]
        hTq_ts = [
            big.tile([128, SLAB], FP16, name=f"hTq{i}", tag=f"hTq{i}") for i in range(8)
        ]
        qT_sb = big.tile([128, 8 * SLAB], FP16)        # d-tile major
        ktT_sb = big.tile([128, 8 * S], FP16)          # d-tile major
        v_sb = big.tile([128, KT * NH * VW], BF16)     # kt major, per-head 65
        pen_sb = big.tile([128, KT * SLAB], BF16)      # kt major

        # DMA issue order (rings are FIFO): dt0 K/Q weights + first hidden
        # chunks feed the PE earliest; the descriptor-heavy broadcast consts
        # go next (penalty path starts ~15us in); bulk hidden after.
        def load_wkq(dt):
            wkb = wk_pool.tile([128, 8, 128], FP16, tag="wkb", name="wkb")
            wqb = wk_pool.tile([128, 8, 128], FP16, tag="wqb", name="wqb")
            nc.sync.dma_start(wkb[:, :, :], _wview(Wk, dt, 128))
            nc.sync.dma_start(wqb[:, :, :], _wview(Wq, dt, 128))
            wkts = [wkb[:, ht, :] for ht in range(8)]
            wqts = [wqb[:, ht, :] for ht in range(8)]
            return (wkts, wqts)

        wkq01 = [load_wkq(0)]
        for ht in range(2):
            nc.sync.dma_start(hT_ts[ht][:, :], hT[ht * 128 : ht * 128 + 128, :])

        idx_pm = consts.tile([128, KT], F32)
        nc.sync.dma_start(idx_pm[:, :], _pm_view(idx, KT))
        z_pm = consts.tile([128, KT], F32)
        nc.sync.dma_start(z_pm[:, :], _pm_view(zk, KT))
        Bq3 = consts.tile([128, 3 * SLAB], BF16)
        nc.sync.dma_start(Bq3[:, :], _bcast(qv3, 128, 3 * SLAB))
        Bq2 = consts.tile([128, 2 * SLAB], F32)
        nc.sync.dma_start(Bq2[:, :], _bcast(qv2, 128, 2 * SLAB))
        BZ = Bq3[:, 0:SLAB]
        BnegZ = Bq3[:, SLAB : 2 * SLAB]
        BA = Bq3[:, 2 * SLAB : 3 * SLAB]
        Balpha = Bq2[:, 0:SLAB]
        Bbeta = Bq2[:, SLAB : 2 * SLAB]
        bk_sb = consts.tile([128, 8], F32)
        nc.sync.dma_start(bk_sb[:, :], _pm_view(bkv, 8))
        bq_sb = consts.tile([128, 8], F32)
        nc.sync.dma_start(bq_sb[:, :], _pm_view(bqv, 8))
        bvp_sb = consts.tile([VW, NH], F32)
        nc.sync.dma_start(bvp_sb[:, :], bvp[:, :])

        for ht in range(2, 8):
            nc.sync.dma_start(hT_ts[ht][:, :], hT[ht * 128 : ht * 128 + 128, :])
            nc.sync.dma_start(
                hTq_ts[ht - 2][:, :], hTq[(ht - 2) * 128 : (ht - 1) * 128, :]
            )
        for ht in range(6, 8):
            nc.sync.dma_start(hTq_ts[ht][:, :], hTq[ht * 128 : ht * 128 + 128, :])
        wkq01.append(load_wkq(1))

        nidx_pm = consts.tile([128, KT], F32)
        nc.vector.tensor_scalar(nidx_pm[:, :], idx_pm[:, :], -1.0, None, OP.mult)
        negz_pm = consts.tile([128, KT], F32)
        nc.vector.tensor_scalar(negz_pm[:, :], z_pm[:, :], -1.0, None, OP.mult)
        a_pm = consts.tile([128, KT], F32)
        nc.vector.tensor_scalar(a_pm[:, :], z_pm[:, :], -1.0, 1.0, OP.mult, OP.add)

        penw = ctx.enter_context(tc.tile_pool(name="penw", bufs=2))
        epool = ctx.enter_context(tc.tile_pool(name="ep", bufs=8))
        cpool = ctx.enter_context(tc.tile_pool(name="cp", bufs=2))

        # ---- penalties pen^T [128 keys x 512 queries] per key-tile ----
        # Runs on DVE from t=0 (only needs the broadcast tiles), concurrent
        # with the PE projection work below.
        for kt in range(KT):
            aj = a_pm[:, kt : kt + 1]
            nzj = negz_pm[:, kt : kt + 1]
            jp = idx_pm[:, kt : kt + 1]
            njp = nidx_pm[:, kt : kt + 1]
            # r1/r2 are relu(bias + x) -> scalar ACT, freeing DVE cycles
            r1 = penw.tile([128, SLAB], BF16, tag="r1")
            nc.scalar.activation(r1[:, :], BnegZ, AF.Relu, bias=aj)
            r2 = penw.tile([128, SLAB], BF16, tag="r2")
            nc.scalar.activation(r2[:, :], BZ, AF.Relu, bias=nzj)
            t = penw.tile([128, SLAB], BF16, tag="t")
            nc.vector.tensor_mul(t[:, :], BA, r1[:, :])
            # u = (r2 - 1) * z_i  (so res = t - u = a*r1 + z*(1-r2))
            u = penw.tile([128, SLAB], BF16, tag="u")
            nc.vector.scalar_tensor_tensor(
                u[:, :], r2[:, :], 1.0, BZ, OP.subtract, OP.mult
            )
            res = penw.tile([128, SLAB], BF16, tag="res")
            nc.vector.tensor_sub(res[:, :], t[:, :], u[:, :])
            # scope = clip(min(alpha+j, beta-j), 0, 1)
            s1 = penw.tile([128, SLAB], F32, tag="s1")
            nc.vector.tensor_scalar(s1[:, :], Balpha, jp, 1.0, OP.add, OP.min)
            sc = penw.tile([128, SLAB], F32, tag="sc")
            nc.vector.scalar_tensor_tensor(
                sc[:, :], Bbeta, njp, s1[:, :], OP.add, OP.min
            )
            scb = penw.tile([128, SLAB], BF16, tag="scb")
            nc.vector.tensor_scalar(scb[:, :], sc[:, :], 0.0, None, OP.max)
            nc.vector.tensor_mul(
                pen_sb[:, kt * SLAB : (kt + 1) * SLAB], res[:, :], scb[:, :]
            )

        # ---- V projection: tv-major per d-half so PV can chase it ----
        ones_view = v_sb[:, :].rearrange("p (k c) -> p k c", c=VW)[:, :, HD : HD + 1]
        nc.gpsimd.memset(ones_view, 1.0)

        def v_wt_load(vd):
            wvb = wv_pool.tile([128, 8, 512], FP16, tag="wvb", name="wvb")
            nc.sync.dma_start(wvb[:, :, :], _wview(Wv, vd, 512))
            return [wvb[:, ht, :] for ht in range(8)]

        def v_unit(wts, vd, tv, dve_copy=False):
            ps = psp.tile([128, 512], F32, tag="ps", name="psv")
            for ht in range(8):
                nc.tensor.matmul(
                    ps[:, :],
                    hT_ts[ht][:, tv * 128 : tv * 128 + 128],
                    wts[ht][:, :],
                    start=(ht == 0),
                    stop=(ht == 7),
                )
            base = tv * NH * VW + vd * 8 * VW
            dst = v_sb[:, base : base + 8 * VW].rearrange(
                "p (h c) -> p h c", c=VW
            )[:, :, 0:HD]
            src = ps[:, :].rearrange("p (h c) -> p h c", c=HD)
            if dve_copy:
                nc.vector.tensor_scalar(dst, src, 0.0, None, OP.add)
            else:
                nc.scalar.copy(dst, src)

        # ---- K^T and Q^T per d-tile ----
        def _copy_ps(dst, ps, bias_ap, use_dve):
            if use_dve:
                # DVE: add per-partition bias then cast
                nc.vector.tensor_scalar(dst, ps, bias_ap, None, OP.add)
            else:
                nc.scalar.activation(dst, ps, AF.Identity, bias=bias_ap)

        def kq_proj_units(dt, copies_on_scalar=False):
            wkts, wqts = load_wkq(dt)

            def k_unit(tt, use_dve):
                ps = psp.tile([128, 512], F32, tag="ps", name="psk")
                for ht in range(8):
                    nc.tensor.matmul(
                        ps[:, :],
                        wkts[ht][:, :],
                        hT_ts[ht][:, tt * 512 : (tt + 1) * 512],
                        start=(ht == 0),
                        stop=(ht == 7),
                    )
                _copy_ps(
                    ktT_sb[:, dt * S + tt * 512 : dt * S + (tt + 1) * 512],
                    ps[:, :],
                    bk_sb[:, dt : dt + 1],
                    use_dve,
                )

            def q_unit(use_dve):
                ps = psp.tile([128, SLAB], F32, tag="ps", name="psq")
                for ht in range(8):
                    nc.tensor.matmul(
                        ps[:, :],
                        wqts[ht][:, :],
                        hTq_ts[ht][:, :],
                        start=(ht == 0),
                        stop=(ht == 7),
                    )
                _copy_ps(
                    qT_sb[:, dt * SLAB : (dt + 1) * SLAB],
                    ps[:, :],
                    bq_sb[:, dt : dt + 1],
                    use_dve,
                )

            kdve = not copies_on_scalar
            units = [lambda tt=tt: k_unit(tt, kdve) for tt in range(4)]
            units.append(lambda: q_unit(False))
            return units

        # Pre-attention PE work: K/Q for dt 0,1 with the contraction loop
        # outermost (ht) so the matmuls chase the hidden-state DMA tile by
        # tile instead of waiting for the full 4MB load. Needs 5 live PSUM
        # accumulators (4 K slabs + Q). Copies on scalar so the DVE stays
        # exclusively on penalties.
        with tc.tile_pool(name="psk5", bufs=1, space="PSUM") as psk5:
            # K for dt0 then dt1 chase the hT DMA stream; the Q passes go
            # last (their hTq tiles land after the hT bulk)
            def k_pass(dt):
                wkts, _ = wkq01[dt]
                kps = [
                    psk5.tile([128, 512], F32, tag=f"kp{i}", name=f"kp{i}")
                    for i in range(4)
                ]
                for ht in range(8):
                    for tt in range(4):
                        nc.tensor.matmul(
                            kps[tt][:, :],
                            wkts[ht][:, :],
                            hT_ts[ht][:, tt * 512 : (tt + 1) * 512],
                            start=(ht == 0),
                            stop=(ht == 7),
                        )
                for tt in range(4):
                    nc.scalar.activation(
                        ktT_sb[:, dt * S + tt * 512 : dt * S + (tt + 1) * 512],
                        kps[tt][:, :],
                        AF.Identity,
                        bias=bk_sb[:, dt : dt + 1],
                    )

            def q_pass(dt):
                _, wqts = wkq01[dt]
                qps = psk5.tile([128, SLAB], F32, tag="qp", name="qp")
                for ht in range(8):
                    nc.tensor.matmul(
                        qps[:, :],
                        wqts[ht][:, :],
                        hTq_ts[ht][:, :],
                        start=(ht == 0),
                        stop=(ht == 7),
                    )
                nc.scalar.activation(
                    qT_sb[:, dt * SLAB : (dt + 1) * SLAB],
                    qps[:, :],
                    AF.Identity,
                    bias=bq_sb[:, dt : dt + 1],
                )

            k_pass(0)
            k_pass(1)
            q_pass(0)
            q_pass(1)
        # ---- attention: per-group filler = K/Q proj for dt g+2 plus the
        # second V half spread over groups 0..3. Groups 6||7 run as a merged
        # software-pipelined pair after the unit PSUM pool closes.
        with (
            tc.tile_pool(name="pss", bufs=2, space="PSUM") as pss,
            tc.tile_pool(name="psv2", bufs=1, space="PSUM") as psv2,
        ):
          with tc.tile_pool(name="psp", bufs=2, space="PSUM") as psp:
            v0_wts = v_wt_load(0)
            for tv in range(KT):
                v_unit(v0_wts, 0, tv)
            v1_wts = None
            for g in range(6):
                  h0, h1 = 2 * g, 2 * g + 1
                  units = kq_proj_units(g + 2)
                  if g == 0:
                      v1_wts = v_wt_load(1)
                  if g < 4:
                      # spread the 16 second-half V units over groups 0..3
                      for tv in range(4 * g, 4 * g + 4):
                          units.append(lambda tv=tv: v_unit(v1_wts, 1, tv))
                      slots = {1, 2, 5, 8, 9, 11, 13, 14}
                  else:
                      slots = {2, 5, 8, 11, 14}
                  pv0 = psv2.tile([VW, 512], F32, tag="pv0")
                  pv1 = psv2.tile([VW, 512], F32, tag="pv1")
                  def pv_mms(kt, e):
                      nc.tensor.matmul(
                          pv0,
                          v_sb[:, kt * NH * VW + h0 * VW : kt * NH * VW + (h0 + 1) * VW],
                          e[:, 0:512],
                          start=(kt == 0),
                          stop=(kt == KT - 1),
                      )
                      nc.tensor.matmul(
                          pv1,
                          v_sb[:, kt * NH * VW + h1 * VW : kt * NH * VW + (h1 + 1) * VW],
                          e[:, 512:1024],
                          start=(kt == 0),
                          stop=(kt == KT - 1),
                      )

                  e_q = []
                  lag = 3
                  for kt in range(KT):
                      sp = pss.tile([128, 1024], F32, tag="sp")
                      nc.tensor.matmul(
                          sp[:, 0:512],
                          ktT_sb[0:64, g * S + kt * 128 : g * S + kt * 128 + 128],
                          qT_sb[0:64, g * SLAB : (g + 1) * SLAB],
                          start=True,
                          stop=True,
                          tile_position=(0, 0),
                      )
                      nc.tensor.matmul(
                          sp[:, 512:1024],
                          ktT_sb[64:128, g * S + kt * 128 : g * S + kt * 128 + 128],
                          qT_sb[64:128, g * SLAB : (g + 1) * SLAB],
                          start=True,
                          stop=True,
                          tile_position=(64, 0),
                      )
                      # PV for a past kt whose E is ready; keeps the in-order
                      # PE queue from head-of-line blocking on the exp/mul chain
                      if e_q and len(e_q) > lag:
                          pv_mms(*e_q.pop(0))
                      if kt in slots and units:
                          units.pop(0)()
                      e = epool.tile([128, 1024], BF16, tag="e")
                      nc.scalar.activation(
                          e[:, :], sp[:, :], AF.Exp, scale=1.0 / math.sqrt(HD)
                      )
                      pen1 = pen_sb[:, kt * SLAB : (kt + 1) * SLAB]
                      # single strided-broadcast mul: one sem hop per kt on
                      # the chain (vs two contiguous ops); all on DVE
                      pen_b = AP(
                          tensor=pen1.tensor, offset=pen1.offset,
                          ap=[pen1.ap[0], [0, 2], pen1.ap[1]],
                      )
                      e_view = e[:, :].rearrange("p (r n) -> p r n", r=2)
                      nc.vector.tensor_mul(e_view, e_view, pen_b)
                      e_q.append((kt, e))
                  for kt_e in e_q:
                      pv_mms(*kt_e)
                  for u in units:
                      u()
                  for h, pv in ((h0, pv0), (h1, pv1)):
                      ctxT = cpool.tile([VW, 512], F32, tag="ctxT")
                      nc.vector.tensor_scalar(
                          ctxT[:, :], pv[:, :], bvp_sb[:, h : h + 1], None, OP.add
                      )
                      nc.sync.dma_start(out[h * VW : (h + 1) * VW, :], ctxT[:, :])

          # ---- merged tail: groups 6 and 7 interleaved kt-by-kt so the
          # exp->mul chain of one hides the PE work of the other ----
          with tc.tile_pool(name="psv3", bufs=1, space="PSUM") as psv3:
            pvt = {}
            for gg, pool in ((6, psv2), (7, psv3)):
                pvt[gg] = (
                    pool.tile([VW, 512], F32, tag="pv0", name="pv0"),
                    pool.tile([VW, 512], F32, tag="pv1", name="pv1"),
                )

            def pv_mms_t(g, kt, e):
                pv0, pv1 = pvt[g]
                hh0, hh1 = 2 * g, 2 * g + 1
                nc.tensor.matmul(
                    pv0,
                    v_sb[:, kt * NH * VW + hh0 * VW : kt * NH * VW + (hh0 + 1) * VW],
                    e[:, 0:512],
                    start=(kt == 0),
                    stop=(kt == KT - 1),
                )
                nc.tensor.matmul(
                    pv1,
                    v_sb[:, kt * NH * VW + hh1 * VW : kt * NH * VW + (hh1 + 1) * VW],
                    e[:, 512:1024],
                    start=(kt == 0),
                    stop=(kt == KT - 1),
                )

            eq = []
            for kt in range(KT):
                for g in (6, 7):
                    sp = pss.tile([128, 1024], F32, tag="sp")
                    nc.tensor.matmul(
                        sp[:, 0:512],
                        ktT_sb[0:64, g * S + kt * 128 : g * S + kt * 128 + 128],
                        qT_sb[0:64, g * SLAB : (g + 1) * SLAB],
                        start=True,
                        stop=True,
                        tile_position=(0, 0),
                    )
                    nc.tensor.matmul(
                        sp[:, 512:1024],
                        ktT_sb[64:128, g * S + kt * 128 : g * S + kt * 128 + 128],
                        qT_sb[64:128, g * SLAB : (g + 1) * SLAB],
                        start=True,
                        stop=True,
                        tile_position=(64, 0),
                    )
                    if eq and len(eq) > 3:
                        pv_mms_t(*eq.pop(0))
                    e = epool.tile([128, 1024], BF16, tag="e")
                    nc.scalar.activation(
                        e[:, :], sp[:, :], AF.Exp, scale=1.0 / math.sqrt(HD)
                    )
                    pen1 = pen_sb[:, kt * SLAB : (kt + 1) * SLAB]
                    pen_b = AP(
                        tensor=pen1.tensor, offset=pen1.offset,
                        ap=[pen1.ap[0], [0, 2], pen1.ap[1]],
                    )
                    e_view = e[:, :].rearrange("p (r n) -> p r n", r=2)
                    nc.vector.tensor_mul(e_view, e_view, pen_b)
                    eq.append((g, kt, e))
            for item in eq:
                pv_mms_t(*item)
            for g in (6, 7):
                for h, pv in ((2 * g, pvt[g][0]), (2 * g + 1, pvt[g][1])):
                    ctxT = cpool.tile([VW, 512], F32, tag="ctxT")
                    if h % 2 == 0:
                        nc.scalar.activation(
                            ctxT[:, :], pv[:, :], AF.Identity,
                            bias=bvp_sb[:, h : h + 1],
                        )
                    else:
                        nc.vector.tensor_scalar(
                            ctxT[:, :], pv[:, :], bvp_sb[:, h : h + 1], None, OP.add
                        )
                    nc.sync.dma_start(out[h * VW : (h + 1) * VW, :], ctxT[:, :])


_NC_CACHE = None


def _get_nc():
    global _NC_CACHE
    if _NC_CACHE is None:
        _NC_CACHE = build_nc()
    return _NC_CACHE


def _prep_inputs(hidden_states, Wq, bq, Wk, bk, Wv, bv, Wg, bg):
    f16 = np.float16
    bf16 = ml_dtypes.bfloat16
    hidden_states = np.asarray(hidden_states, np.float32)

    def tile_w(W, width):
        # [1024, H] -> [H//width, 1025, width] contiguous blocks (row 1024 pad)
        Wa = np.vstack([np.asarray(W, np.float32), np.zeros((1, H), np.float32)])
        n = H // width
        return np.ascontiguousarray(
            Wa.reshape(H + 1, n, width).transpose(1, 0, 2)
        ).astype(f16)

    Wq_a = tile_w(Wq, 128)
    Wk_a = tile_w(Wk, 128)
    Wv_a = tile_w(Wv, 512)
    bq_v = np.asarray(bq, np.float32)
    bk_v = np.asarray(bk, np.float32)
    bv_v = np.asarray(bv, np.float32)
    bvp_a = np.zeros((VW, NH), np.float32)
    bvp_a[0:HD, :] = bv_v.reshape(NH, HD).T
    idx_all = np.arange(S, dtype=np.float32)

    # host-side granularity gate (f64): z = sigmoid(h @ Wg + bg), [B, S]
    Wg_f = np.asarray(Wg, np.float64).reshape(H)
    bg_f = float(np.asarray(bg, np.float64).reshape(()))
    z_all = 1.0 / (1.0 + np.exp(-(hidden_states.astype(np.float64) @ Wg_f + bg_f)))

    in_maps = []
    for c in range(NC):
        b = c // 4
        q0 = (c % 4) * SLAB
        hT_f = hidden_states[b].T  # [H, S]
        hT_full = hT_f.astype(f16)
        hTq = hT_f[:, q0 : q0 + SLAB].astype(f16)
        zq = z_all[b, q0 : q0 + SLAB]
        w = np.exp((1.0 - zq) * LN_BASE)
        iq = idx_all[q0 : q0 + SLAB].astype(np.float64)
        in_maps.append(
            {
                "hT": hT_full,
                "hTq": np.ascontiguousarray(hTq),
                "Wq": Wq_a,
                "Wk": Wk_a,
                "Wv": Wv_a,
                "bqv": bq_v,
                "bkv": bk_v,
                "bvp": bvp_a,
                "idx": idx_all,
                "zk": z_all[b].astype(np.float32),
                "qv3": np.concatenate([zq, -zq, 1.0 - zq]).astype(bf16),
                "qv2": np.concatenate(
                    [w + 2.0 - iq, w + 2.0 + iq]
                ).astype(np.float32),
            }
        )
    return in_maps


def kernel(**inputs) -> np.ndarray:
    nc = _get_nc()
    in_maps = _prep_inputs(**inputs)
    res = run_bass_kernel_spmd(nc, in_maps, core_ids=list(range(NC)))
    out = np.empty((B, S, H), np.float32)
    for c in range(NC):
        b = c // 4
        q0 = (c % 4) * SLAB
        ctx_t = res.results[c]["out"].reshape(NH, VW, SLAB)
        vals = ctx_t[:, 0:HD, :]            # [NH, 64, SLAB]
        l = ctx_t[:, HD, :]                 # [NH, SLAB]
        ctx = (vals / l[:, None, :]).transpose(2, 0, 1)  # [SLAB, NH, 64]
        out[b, q0 : q0 + SLAB, :] = ctx.reshape(SLAB, H)
    return out



# revision 2
# speedup vs baseline: 1.4601x; 1.4601x over previous
"""Trainium2 Bass kernel: BERT self-attention with granularity-gated sparse
penalties, exploiting the data-dependent banded mask.

Math: softmax(S/8 + log(max(pen,1e-32))) == pen*exp(S/8) / sum(pen*exp(S/8)).
pen = res*scope with scope = clip(w_q+2 - |i-j|, 0, 1): a per-query BAND of
half-width w_q+2 = (S-2)^(1-z_q)+2.  ~88% of (q,k) pairs are exactly masked.

Sharding (8 cores): core c -> batch c//4, heads 4*(c%4)..+4 (dims 256*(c%4)).
K/V/Q projections computed only for the core's 256 dims (no redundancy).

Sparsity schedule (host, data-dependent, compiled per input):
 - queries sorted by band-width rank into chunks of CW=256 (index-sorted
   inside); per (chunk, key-tile kt) the active queries form a contiguous
   segment [A,B) after monotone closure (A,B non-decreasing in kt).
 - segments are the UNION over both batches -> identical program structure
   for all 8 cores (SPMD); extra columns self-zero via that batch's pen.
 - scores/exp/pen-mul/PV run only on segment columns (~25% of dense).

Per (pair of heads, group of 1024 queries): kt-loop; scores [128k x W] into
sp PSUM ([h0 bank | h1 bank]); one exp (ACT) per window; one pen-mul (DVE,
broadcast over the 2 heads); PV accumulates V^T@E into pv [65, 1024] PSUM
using per-byte pending-zero semantics (split at B_prev, bank start/stop at
first/last touch).  l = ones column 65 of V; host divides and un-permutes.

Penalties are host-precomputed (0.03% of FLOPs) packed [128, PENW] bf16 in
window order and DMA'd; gate z computed host-side in f64.
"""

import math

import ml_dtypes
import numpy as np

import concourse.bass as bass
import concourse.tile as tile
from concourse import bacc, mybir
from concourse.bass import AP
from concourse.bass_utils import run_bass_kernel_spmd

F32 = mybir.dt.float32
BF16 = mybir.dt.bfloat16
FP16 = mybir.dt.float16
AF = mybir.ActivationFunctionType
OP = mybir.AluOpType

B, S, H = 2, 2048, 1024
NH, HD = 16, 64
NC = 8
KT = 16            # key tiles of 128
CW = 256           # closure chunk width (queries)
NCH = S // CW      # 8 chunks
VW = HD + 1        # 65: V dims + ones column
GW = 1024          # query group width (pv tile)
WCAP = 512         # max window cols per head (one PSUM bank)
LAG = 3
LN_BASE = float(np.log(np.float32(S - 2)))


# ---------------------------------------------------------------- planning

def _gate_z(hidden, Wg, bg):
    Wg_f = np.asarray(Wg, np.float64).reshape(H)
    bg_f = float(np.asarray(bg, np.float64).reshape(()))
    pre = np.asarray(hidden, np.float64).reshape(B * S, H) @ Wg_f + bg_f
    return (1.0 / (1.0 + np.exp(-pre))).reshape(B, S)


def _make_plan(z):
    """Common (union over batches) sparse schedule + per-batch permutations."""
    idx = np.arange(S)
    t = np.exp((1.0 - z) * LN_BASE) + 2.0   # band half-width per query
    perms = []
    segs_b = np.full((B, NCH, KT, 2), -1, np.int64)
    for b in range(B):
        rank = np.argsort(np.argsort(t[b], kind="stable"), kind="stable")
        perm = np.lexsort((idx, rank // CW))
        perms.append(perm)
        lo = np.clip(np.floor(idx - t[b]) - 1, 0, S - 1).astype(np.int64)
        hi = np.clip(np.ceil(idx + t[b]) + 1, 0, S - 1).astype(np.int64)
        ktlo = lo[perm] // 128
        kthi = hi[perm] // 128
        for c in range(NCH):
            sl = slice(c * CW, (c + 1) * CW)
            kl = np.minimum.accumulate(ktlo[sl][::-1])[::-1]
            kh = np.maximum.accumulate(kthi[sl])
            for kt in range(KT):
                act = np.nonzero((kl <= kt) & (kh >= kt))[0]
                if len(act):
                    segs_b[b, c, kt] = (c * CW + act.min(), c * CW + act.max() + 1)

    # union across batches, then monotone closure (A suffix-min, B prefix-max)
    chunk_segs = []   # per chunk: list of (kt, A, B, Bprev)
    for c in range(NCH):
        A = np.full(KT, 1 << 30, np.int64)
        Bn = np.full(KT, -1, np.int64)
        for b in range(B):
            for kt in range(KT):
                a, bb = segs_b[b, c, kt]
                if a >= 0:
                    A[kt] = min(A[kt], a)
                    Bn[kt] = max(Bn[kt], bb)
        ne = np.nonzero(Bn >= 0)[0]
        k0, k1 = int(ne.min()), int(ne.max())
        for kt in range(k1 - 1, k0 - 1, -1):
            A[kt] = min(A[kt], A[kt + 1])
        for kt in range(k0 + 1, k1 + 1):
            Bn[kt] = max(Bn[kt], Bn[kt - 1])
        segs = []
        for kt in range(k0, k1 + 1):
            if A[kt] < Bn[kt]:
                bprev = int(Bn[kt - 1]) if kt > k0 else int(A[kt])
                bprev = min(max(bprev, int(A[kt])), int(Bn[kt]))
                segs.append((kt, int(A[kt]), int(Bn[kt]), bprev))
        chunk_segs.append(segs)

    # windows per (group, kt): segments of the group's chunks, packed <= WCAP
    sched = [[] for _ in range(2)]      # sched[g] = [(kt, win)], win = dict
    for g in range(2):
        for kt in range(KT):
            segs = []
            for c in range(4 * g, 4 * g + 4):
                for (skt, a, bb, bp) in chunk_segs[c]:
                    if skt == kt:
                        segs.append({"c": c, "A": a, "B": bb, "Bp": bp})
            cur = []
            cw = 0
            for sg in segs:
                w = sg["B"] - sg["A"]
                if cur and cw + w > WCAP:
                    sched[g].append((kt, {"segs": cur, "sw": cw}))
                    cur, cw = [], 0
                sg["off"] = cw
                cur.append(sg)
                cw += w
            if cur:
                sched[g].append((kt, {"segs": cur, "sw": cw}))

    # pen buffer offsets (kt-major layout; order only needs consistency)
    off = 0
    for kt in range(KT):
        for g in range(2):
            for (wkt, win) in sched[g]:
                if wkt == kt:
                    win["pen_off"] = off
                    off += win["sw"]
    penw = off + (-off) % 8

    # PV parts + PSUM bank start/stop flags per (group, half-bank)
    for g in range(2):
        for hb in range(2):
            chunks = {4 * g + 2 * hb, 4 * g + 2 * hb + 1}
            parts_seq = []
            for (kt, win) in sched[g]:
                for sg in win["segs"]:
                    if sg["c"] in chunks:
                        parts = []
                        m = min(sg["Bp"], sg["B"])
                        if sg["A"] < m:
                            parts.append([sg["A"], m, False, False])
                        f0 = max(sg["A"], sg["Bp"])
                        if f0 < sg["B"]:
                            parts.append([f0, sg["B"], False, False])
                        sg["parts"] = parts
                        parts_seq.extend(parts)
            parts_seq[0][2] = True    # start on first touch of this bank
            parts_seq[-1][3] = True   # stop on last touch

    sig = (penw, tuple(
        (g, kt, tuple((sg["c"], sg["A"], sg["B"], sg["Bp"]) for sg in win["segs"]))
        for g in range(2) for (kt, win) in sched[g]
    ))
    return {"perms": perms, "sched": sched, "penw": penw, "t": t, "sig": sig}


# ---------------------------------------------------------------- device

def _pm_view(ap_1d, n_free):
    return AP(tensor=ap_1d.tensor, offset=ap_1d.offset, ap=[[1, 128], [128, n_free]])


def _wview(w2d, dt, width):
    return AP(
        tensor=w2d.tensor,
        offset=w2d.offset + dt * 8 * 128 * width,
        ap=[[width, 128], [128 * width, 8], [1, width]],
    )


def build_nc(plan):
    nc = bacc.Bacc("TRN2", target_bir_lowering=False, debug=False)
    penw = plan["penw"]
    hT = nc.dram_tensor("hT", [H, S], FP16, kind="ExternalInput").ap()
    hTq = nc.dram_tensor("hTq", [H, S], FP16, kind="ExternalInput").ap()
    Wk = nc.dram_tensor("Wk", [2 * 8 * 128, 128], FP16, kind="ExternalInput").ap()
    Wq = nc.dram_tensor("Wq", [2 * 8 * 128, 128], FP16, kind="ExternalInput").ap()
    Wv = nc.dram_tensor("Wv", [8 * 128, 256], FP16, kind="ExternalInput").ap()
    bkq = nc.dram_tensor("bkq", [4 * 128], F32, kind="ExternalInput").ap()
    bvp = nc.dram_tensor("bvp", [VW, 4], F32, kind="ExternalInput").ap()
    pen = nc.dram_tensor("pen", [128, penw], BF16, kind="ExternalInput").ap()
    out = nc.dram_tensor("out", [4 * VW, S], F32, kind="ExternalOutput").ap()

    with tile.TileContext(nc) as tc:
        _body(tc, nc, plan, hT, hTq, Wk, Wq, Wv, bkq, bvp, pen, out)
    nc.compile()
    return nc


def _body(tc, nc, plan, hT, hTq, Wk, Wq, Wv, bkq, bvp, pen, out):
    import contextlib

    penw = plan["penw"]
    sched = plan["sched"]
    ctx = contextlib.ExitStack()
    with ctx:
        big = ctx.enter_context(tc.tile_pool(name="big", bufs=1))
        wpool = ctx.enter_context(tc.tile_pool(name="wp", bufs=1))
        epool = ctx.enter_context(tc.tile_pool(name="ep", bufs=6))
        cpool = ctx.enter_context(tc.tile_pool(name="cp", bufs=4))

        hT_ts = [big.tile([128, S], FP16, name=f"hT{i}", tag=f"hT{i}") for i in range(8)]
        hTq_ts = [
            big.tile([128, S], FP16, name=f"hTq{i}", tag=f"hTq{i}") for i in range(8)
        ]
        ktT = big.tile([128, 2 * S], FP16)
        qT = big.tile([128, 2 * S], FP16)
        v_sb = big.tile([128, KT * 4 * VW], BF16)
        pen_sb = big.tile([128, penw], BF16)
        bkq_sb = big.tile([128, 4], F32)
        bvp_sb = big.tile([VW, 4], F32)

        # --- DMA issue order: K weights + hT first (K proj chases), then
        # hTq (Q proj), then V weights / pen / consts.
        wkb = wpool.tile([128, 2, 8, 128], FP16, name="wkb")
        wqb = wpool.tile([128, 2, 8, 128], FP16, name="wqb")
        for dt in range(2):
            nc.sync.dma_start(wkb[:, dt, :, :], _wview(Wk, dt, 128))
        for ht in range(8):
            nc.sync.dma_start(hT_ts[ht][:, :], hT[ht * 128 : ht * 128 + 128, :])
        wvb = wpool.tile([128, 8, 256], FP16, name="wvb")
        nc.sync.dma_start(wvb[:, :, :], _wview(Wv, 0, 256))
        for dt in range(2):
            nc.sync.dma_start(wqb[:, dt, :, :], _wview(Wq, dt, 128))
        for ht in range(8):
            nc.sync.dma_start(hTq_ts[ht][:, :], hTq[ht * 128 : ht * 128 + 128, :])
        nc.sync.dma_start(pen_sb[:, :], pen[:, :])
        nc.sync.dma_start(bkq_sb[:, :], _pm_view(bkq, 4))
        nc.sync.dma_start(bvp_sb[:, :], bvp[:, :])

        ones_view = v_sb[:, :].rearrange("p (k c) -> p k c", c=VW)[:, :, HD : HD + 1]
        nc.gpsimd.memset(ones_view, 1.0)

        # --- K projection: ktT[dims 128/dt, 2048 keys], chases hT DMA
        with tc.tile_pool(name="pk", bufs=2, space="PSUM") as pk:
            for dt in range(2):
                kps = [
                    pk.tile([128, 512], F32, tag=f"kp{i}", name=f"kp{i}")
                    for i in range(4)
                ]
                for ht in range(8):
                    for tt in range(4):
                        nc.tensor.matmul(
                            kps[tt][:, :],
                            wkb[:, dt, ht, :],
                            hT_ts[ht][:, tt * 512 : (tt + 1) * 512],
                            start=(ht == 0),
                            stop=(ht == 7),
                        )
                for tt in range(4):
                    nc.scalar.activation(
                        ktT[:, dt * S + tt * 512 : dt * S + (tt + 1) * 512],
                        kps[tt][:, :],
                        AF.Identity,
                        bias=bkq_sb[:, dt : dt + 1],
                    )

        # --- V projection: v_sb[128 keys, kt*260 + h*65 (+64: ones)]
        with tc.tile_pool(name="pV", bufs=4, space="PSUM") as pV:
            for tv in range(KT):
                ps = pV.tile([128, 256], F32, tag="vps", name="vps")
                for ht in range(8):
                    nc.tensor.matmul(
                        ps[:, :],
                        hT_ts[ht][:, tv * 128 : tv * 128 + 128],
                        wvb[:, ht, :],
                        start=(ht == 0),
                        stop=(ht == 7),
                    )
                base = tv * 4 * VW
                dst = v_sb[:, base : base + 4 * VW].rearrange(
                    "p (h c) -> p h c", c=VW
                )[:, :, 0:HD]
                src = ps[:, :].rearrange("p (h c) -> p h c", c=HD)
                nc.vector.tensor_scalar(dst, src, 0.0, None, OP.add)

        # --- Q projection: qT[dims, 2048 permuted queries]
        with tc.tile_pool(name="pQ", bufs=1, space="PSUM") as pQ:
            qps = [
                [pQ.tile([128, 512], F32, tag=f"qp{d}{t}", name=f"qp{d}{t}")
                 for t in range(4)]
                for d in range(2)
            ]
            for ht in range(8):
                for dt in range(2):
                    for tt in range(4):
                        nc.tensor.matmul(
                            qps[dt][tt][:, :],
                            wqb[:, dt, ht, :],
                            hTq_ts[ht][:, tt * 512 : (tt + 1) * 512],
                            start=(ht == 0),
                            stop=(ht == 7),
                        )
            for dt in range(2):
                for tt in range(4):
                    nc.scalar.activation(
                        qT[:, dt * S + tt * 512 : dt * S + (tt + 1) * 512],
                        qps[dt][tt][:, :],
                        AF.Identity,
                        bias=bkq_sb[:, 2 + dt : 3 + dt],
                    )

        # --- attention: pairs sequential; per (pair, group) a kt loop
        with (
            tc.tile_pool(name="sp", bufs=2, space="PSUM") as spp,
            tc.tile_pool(name="pvp", bufs=1, space="PSUM") as pvp,
        ):
            for pair in range(2):
                for g in range(2):
                    pv = [
                        pvp.tile([VW, GW], F32, tag=f"pv{h}", name=f"pv{h}")
                        for h in range(2)
                    ]

                    def emit_pv(kt, win, e, pair=pair, g=g, pv=pv):
                        sw = win["sw"]
                        for sg in win["segs"]:
                            for h in range(2):
                                lhs = v_sb[
                                    :,
                                    kt * 4 * VW + (2 * pair + h) * VW :
                                    kt * 4 * VW + (2 * pair + h) * VW + VW,
                                ]
                                for (r0, r1, st, sp_) in sg["parts"]:
                                    eoff = h * sw + sg["off"] + (r0 - sg["A"])
                                    nc.tensor.matmul(
                                        pv[h][:, r0 - GW * g : r1 - GW * g],
                                        lhs,
                                        e[:, eoff : eoff + (r1 - r0)],
                                        start=st,
                                        stop=sp_,
                                    )

                    wq_q = []
                    for (kt, win) in sched[g]:
                        sw = win["sw"]
                        sp = spp.tile([128, 1024], F32, tag="sp")
                        nseg = len(win["segs"])
                        for si, sg in enumerate(win["segs"]):
                            a, bb = sg["A"], sg["B"]
                            for h in range(2):
                                nc.tensor.matmul(
                                    sp[:, h * 512 + sg["off"] : h * 512 + sg["off"] + (bb - a)],
                                    ktT[
                                        h * 64 : h * 64 + 64,
                                        pair * S + kt * 128 : pair * S + kt * 128 + 128,
                                    ],
                                    qT[h * 64 : h * 64 + 64, pair * S + a : pair * S + bb],
                                    start=(si == 0),
                                    stop=(si == nseg - 1),
                                    tile_position=(h * 64, 0),
                                )
                        if len(wq_q) > LAG:
                            emit_pv(*wq_q.pop(0))
                        e = epool.tile([128, 1024], BF16, tag="e")
                        e_view = e[:, 0 : 2 * sw].rearrange("p (r n) -> p r n", r=2)
                        sp_view = sp[:, :].rearrange("p (r n) -> p r n", r=2)[:, :, 0:sw]
                        nc.scalar.activation(
                            e_view, sp_view, AF.Exp, scale=1.0 / math.sqrt(HD)
                        )
                        p1 = pen_sb[:, win["pen_off"] : win["pen_off"] + sw]
                        pen_b = AP(
                            tensor=p1.tensor, offset=p1.offset,
                            ap=[p1.ap[0], [0, 2], p1.ap[1]],
                        )
                        nc.vector.tensor_mul(e_view, e_view, pen_b)
                        wq_q.append((kt, win, e))
                    for item in wq_q:
                        emit_pv(*item)

                    for h in range(2):
                        hg = 2 * pair + h
                        ctxT = cpool.tile([VW, GW], F32, tag="ctxT")
                        nc.vector.tensor_scalar(
                            ctxT[:, :], pv[h][:, :], bvp_sb[:, hg : hg + 1],
                            None, OP.add,
                        )
                        nc.sync.dma_start(
                            out[hg * VW : (hg + 1) * VW, g * GW : (g + 1) * GW],
                            ctxT[:, :],
                        )


# ---------------------------------------------------------------- host

_NC_CACHE = {}


def _get_nc(plan):
    key = hash(plan["sig"])
    if key not in _NC_CACHE:
        _NC_CACHE[key] = build_nc(plan)
    return _NC_CACHE[key]


def _build_pen(plan, z):
    """Packed penalties [B][128, PENW] bf16 in window layout."""
    t = plan["t"]
    pens = []
    for b in range(B):
        perm = plan["perms"][b]
        zb = z[b]
        tb = t[b]
        buf = np.zeros((128, plan["penw"]), np.float64)
        for g in range(2):
            for (kt, win) in plan["sched"][g]:
                j = (kt * 128 + np.arange(128))[:, None]          # keys
                zj = zb[kt * 128 : kt * 128 + 128][:, None]
                off = win["pen_off"]
                for sg in win["segs"]:
                    qs = perm[sg["A"] : sg["B"]]
                    zq = zb[qs][None, :]
                    res = (1.0 - zq) * np.maximum(1.0 - zq - zj, 0.0) + \
                        zq * np.minimum(1.0 - zq + zj, 1.0)
                    scope = np.clip(tb[qs][None, :] - np.abs(qs[None, :] - j), 0.0, 1.0)
                    w = sg["B"] - sg["A"]
                    buf[:, off + sg["off"] : off + sg["off"] + w] = res * scope
        pens.append(buf.astype(ml_dtypes.bfloat16))
    return pens


def _prep_inputs(plan, hidden_states, Wq, bq, Wk, bk, Wv, bv, Wg, bg):
    f16 = np.float16
    hidden = np.asarray(hidden_states, np.float32)
    z = _gate_z(hidden, Wg, bg)
    pens = _build_pen(plan, z)

    Wq_f = np.asarray(Wq, np.float32)
    Wk_f = np.asarray(Wk, np.float32)
    Wv_f = np.asarray(Wv, np.float32)
    bq_f = np.asarray(bq, np.float32)
    bk_f = np.asarray(bk, np.float32)
    bv_f = np.asarray(bv, np.float32)

    in_maps = []
    for c in range(NC):
        b = c // 4
        hg = c % 4
        d0 = 256 * hg
        hT_f = hidden[b].T.astype(f16)                     # [H, S]
        hTq_f = np.ascontiguousarray(hT_f[:, plan["perms"][b]])

        def pack_w(Wf, width):
            # [(dt, ht), 128 rows, width cols] contiguous
            blocks = []
            ndt = 256 // width
            for dt in range(ndt):
                for ht in range(8):
                    blocks.append(
                        Wf[128 * ht : 128 * ht + 128, d0 + width * dt : d0 + width * (dt + 1)]
                    )
            return np.ascontiguousarray(np.concatenate(blocks, 0)).astype(f16)

        bkq_v = np.concatenate(
            [bk_f[d0 : d0 + 256], bq_f[d0 : d0 + 256]]
        ).astype(np.float32)
        bvp_a = np.zeros((VW, 4), np.float32)
        bvp_a[0:HD, :] = bv_f[d0 : d0 + 256].reshape(4, HD).T

        in_maps.append(
            {
                "hT": hT_f,
                "hTq": hTq_f,
                "Wk": pack_w(Wk_f, 128),
                "Wq": pack_w(Wq_f, 128),
                "Wv": pack_w(Wv_f, 256),
                "bkq": bkq_v,
                "bvp": bvp_a,
                "pen": pens[b],
            }
        )
    return in_maps


def _unshard(plan, results):
    out = np.empty((B, S, H), np.float32)
    for c in range(NC):
        b = c // 4
        hg = c % 4
        o = np.asarray(results[c]["out"]).reshape(4, VW, S)
        ctx = o[:, 0:HD, :] / o[:, HD : HD + 1, :]          # [4, 64, S]
        ctx = ctx.transpose(2, 0, 1).reshape(S, 256)        # [S perm, 256]
        out[b][plan["perms"][b], 256 * hg : 256 * hg + 256] = ctx
    return out


def _run(inputs, trace=False):
    z = _gate_z(
        np.asarray(inputs["hidden_states"], np.float32), inputs["Wg"], inputs["bg"]
    )
    plan = _make_plan(z)
    nc = _get_nc(plan)
    in_maps = _prep_inputs(plan, **inputs)
    res = run_bass_kernel_spmd(nc, in_maps, core_ids=list(range(NC)), trace=trace)
    return _unshard(plan, res.results), res


def kernel(**inputs) -> np.ndarray:
    out, _ = _run(inputs)
    return out
